# revision 34
# baseline (speedup 1.0000x reference)
"""Trainium2 Bass kernel for nn_AttentionMeta_58196806861321.

Math (B=1, S=512, D=256):
    k = key + key@Wk + bk ;  q = query + query@Wq + bq ;  v = value + value@Wva + bva
    raw[sk,sq,:]  = k[sk,:] * q[sq,:]
    x             = raw + raw@Wl + bl                  (logits, [Sk,Sq,D])
    xexp          = x * exp(x - max_sq(x))             (swishmax over the QUERY axis)
    scale         = xexp / (sum_sq|xexp| + 1)
    vsum[sq,:]    = sum_sk v[sk,:] * scale[sk,sq,:]
    out           = vsum + vsum@Wvo + bvo

Implementation (per core, Sk sharded 8 x 64 per the key-axis sharding hint):
  * logits^T[dout, sq] = (diag(k_sk) @ (I+Wl))^T @ q^T : a per-key fp16
    VectorE rescale of the folded weight + 4 fp16 PE matmuls per key; the
    quadratic [Sk,Sq,D] tensor is never materialized as such.
  * bl and the exp shift C enter as per-partition biases on the two ScalarE
    streams:  e' = Exp(x_raw + (bl - C)),  xb = Identity(x_raw + bl).
  * swishmax normalizer, algebraically exact vs the reference:
        coeff = v / (sum_sq |x e^{x-C}| + max_sq e^{x-C})
              = v * xexp_true / (sum|xexp_true| + 1)
    max/sum via VectorE tensor_reduce (max over e'; add with
    apply_absolute_value over xexp = xb*e'); column math batched over 8 keys.
  * the weighting/accumulation over sk runs on the PE as diag(coeff) bf16
    matmuls into a PSUM vsum accumulator.
  * the key loop is split in two halves, each draining vsum through the
    (I+Wvo) fp16 matmul (which also transposes [dout,sq] -> [sq,dout], with
    bvo/16 folded per core) into its own bf16 ReduceScatter(add) — the first
    collective overlaps the second half of the compute. Each core returns its
    64-row sq shard; the host concatenates.
"""

import os
import sys

import numpy as np

for _p in ("/opt/trn_rl_repo", "/root/.axon_site/_ro/trn_rl_repo"):
    if os.path.isdir(_p) and _p not in sys.path:
        sys.path.append(_p)

import ml_dtypes  # noqa: E402

import concourse.bacc as bacc  # noqa: E402
import concourse.bass as bass  # noqa: E402
import concourse.tile as tile  # noqa: E402
from concourse import mybir  # noqa: E402
from concourse.bass_utils import run_bass_kernel_spmd  # noqa: E402

F32 = mybir.dt.float32
F16 = mybir.dt.float16
BF16 = mybir.dt.bfloat16
AX = mybir.AxisListType
ALU = mybir.AluOpType
ACTF = mybir.ActivationFunctionType

S = 512
D = 256
N_CORES = 8
SK_LOC = S // N_CORES  # 64 keys per core
GRP = 4  # keys per column-math batch
C_SHIFT = 14.0  # global exp shift; logits peak ~21.8 on this data
MM_DT = F16

_CACHE = {}
NO_CC = False  # test-only: replace the collective with a DMA (TimelineSim)


def _build():
    nc = bacc.Bacc(
        "TRN2",
        target_bir_lowering=False,
        debug=False,
        num_devices=N_CORES,
    )

    qTin = nc.dram_tensor("qTin", [D, S], F32, kind="ExternalInput").ap()
    kTin = nc.dram_tensor("kTin", [D, SK_LOC], F32, kind="ExternalInput").ap()
    vTin = nc.dram_tensor("vTin", [D, SK_LOC], F32, kind="ExternalInput").ap()
    wq = nc.dram_tensor("wq", [D, D], F32, kind="ExternalInput").ap()
    wk = nc.dram_tensor("wk", [D, D], F32, kind="ExternalInput").ap()
    wv = nc.dram_tensor("wv", [D, D], F32, kind="ExternalInput").ap()
    wl = nc.dram_tensor("wl", [D, D], F32, kind="ExternalInput").ap()
    wvo = nc.dram_tensor("wvo", [D, D], F32, kind="ExternalInput").ap()
    bq = nc.dram_tensor("bq", [1, D], F32, kind="ExternalInput").ap()
    bk = nc.dram_tensor("bk", [1, D], F32, kind="ExternalInput").ap()
    bv = nc.dram_tensor("bv", [1, D], F32, kind="ExternalInput").ap()
    blc = nc.dram_tensor("blc", [128, 2], F32, kind="ExternalInput").ap()
    blr = nc.dram_tensor("blr", [2, 128], F32, kind="ExternalInput").ap()
    bvo8 = nc.dram_tensor("bvo8", [1, D], F32, kind="ExternalInput").ap()
    ident = nc.dram_tensor("ident", [128, 128], BF16, kind="ExternalInput").ap()
    out_ext = nc.dram_tensor("out", [SK_LOC, D], F32, kind="ExternalOutput").ap()

    with tile.TileContext(nc) as tc:
        _emit(nc, tc, locals())
    nc.compile()
    return nc


def _emit(nc, tc, io):
    qTin, kTin, vTin = io["qTin"], io["kTin"], io["vTin"]
    wq, wk, wv, wl, wvo = io["wq"], io["wk"], io["wv"], io["wl"], io["wvo"]
    bq, bk, bv, blc, bvo8 = io["bq"], io["bk"], io["bv"], io["blc"], io["bvo8"]
    blr = io["blr"]
    ident, out_ext = io["ident"], io["out_ext"]

    import contextlib

    ctx = contextlib.ExitStack()
    with ctx:
        const = ctx.enter_context(tc.tile_pool(name="const", bufs=1))
        wmod_p = ctx.enter_context(tc.tile_pool(name="wmod", bufs=8))
        x_ps = ctx.enter_context(tc.tile_pool(name="x_ps", bufs=2, space="PSUM"))
        vs_ps = ctx.enter_context(tc.tile_pool(name="vs_ps", bufs=2, space="PSUM"))
        spool = ctx.enter_context(tc.tile_pool(name="spool", bufs=6))
        xpool = ctx.enter_context(tc.tile_pool(name="xpool", bufs=GRP + 2))
        hpool = ctx.enter_context(tc.tile_pool(name="hpool", bufs=3))
        cpool = ctx.enter_context(tc.tile_pool(name="cpool", bufs=4))
        dpool = ctx.enter_context(tc.tile_pool(name="dpool", bufs=8))
        fpool = ctx.enter_context(tc.tile_pool(name="fpool", bufs=4))
        dram = ctx.enter_context(tc.tile_pool(name="dram", bufs=1, space="DRAM"))

        # ---- constants / weights into SBUF ---------------------------------
        qTin_sb = const.tile([128, 2, S], F32)
        kTin_sb = const.tile([128, 2, SK_LOC], F32)
        vTin_sb = const.tile([128, 2, SK_LOC], F32)
        wq_sb = const.tile([128, 2, D], F32)
        wk_sb = const.tile([128, 2, D], F32)
        wv_sb = const.tile([128, 2, D], F32)
        wl_sb = const.tile([128, 2, D], F32)
        wvo_sb = const.tile([128, 2, D], F32)
        bq_sb = const.tile([1, D], F32)
        bk_sb = const.tile([1, D], F32)
        bv_sb = const.tile([1, D], F32)
        blc_sb = const.tile([128, 2], F32)
        blr0_sb = const.tile([1, 128], F32)
        blr1_sb = const.tile([1, 128], F32)
        bvo8_sb = const.tile([1, D], F32)
        nc.sync.dma_start(out=bq_sb, in_=bq)
        nc.sync.dma_start(out=bk_sb, in_=bk)
        nc.sync.dma_start(out=bv_sb, in_=bv)
        nc.sync.dma_start(out=blc_sb, in_=blc)
        nc.sync.dma_start(out=blr0_sb, in_=blr[0:1, :])
        nc.sync.dma_start(out=blr1_sb, in_=blr[1:2, :])
        nc.sync.dma_start(out=bvo8_sb, in_=bvo8)
        ident_sb = const.tile([128, 128], BF16)
        nc.sync.dma_start(out=ident_sb, in_=ident)
        for k in range(2):
            nc.sync.dma_start(out=qTin_sb[:, k, :], in_=qTin[128 * k : 128 * (k + 1), :])
            nc.sync.dma_start(out=kTin_sb[:, k, :], in_=kTin[128 * k : 128 * (k + 1), :])
            nc.sync.dma_start(out=vTin_sb[:, k, :], in_=vTin[128 * k : 128 * (k + 1), :])
            nc.sync.dma_start(out=wq_sb[:, k, :], in_=wq[128 * k : 128 * (k + 1), :])
            nc.sync.dma_start(out=wk_sb[:, k, :], in_=wk[128 * k : 128 * (k + 1), :])
            nc.sync.dma_start(out=wv_sb[:, k, :], in_=wv[128 * k : 128 * (k + 1), :])
            nc.sync.dma_start(out=wl_sb[:, k, :], in_=wl[128 * k : 128 * (k + 1), :])
            nc.sync.dma_start(out=wvo_sb[:, k, :], in_=wvo[128 * k : 128 * (k + 1), :])
        ones_sb = const.tile([1, S], F32)
        nc.vector.memset(ones_sb, 1.0)
        negc_sb = const.tile([128, 1], F32)
        nc.vector.memset(negc_sb, -C_SHIFT)
        blmc_sb = const.tile([128, 2], F32)
        nc.vector.tensor_scalar_add(blmc_sb, blc_sb, -C_SHIFT)

        # fp16 copies for the main-loop matmuls / final linear
        wl_mm = const.tile([128, 2, D], MM_DT)
        wvo_mm = const.tile([128, 2, D], MM_DT)
        bvo8_mm = const.tile([1, D], MM_DT)
        ones_mm = const.tile([1, S], MM_DT)
        for k in range(2):
            nc.vector.tensor_copy(out=wl_mm[:, k, :], in_=wl_sb[:, k, :])
            nc.vector.tensor_copy(out=wvo_mm[:, k, :], in_=wvo_sb[:, k, :])
        nc.vector.tensor_copy(out=bvo8_mm, in_=bvo8_sb)
        nc.vector.tensor_copy(out=ones_mm, in_=ones_sb)

        # ---- PE warm-up: keep the HAM busy while DMAs land ------------------
        warm = const.tile([128, S], MM_DT)
        nc.vector.memset(warm, 0.0)
        wm_ps = x_ps.tile([128, 2, S], F32, tag="x")
        for _ in range(16):
            nc.tensor.matmul(wm_ps[:, 0, :], lhsT=warm[:, 0:128], rhs=warm, start=True, stop=True)

        # ---- prep: qT/kT/vT residual linears (kept transposed) --------------
        qT_sb = const.tile([128, 2, S], MM_DT)
        kT_sb = const.tile([128, 2, SK_LOC], F32)
        vT_sb = const.tile([128, 2, SK_LOC], F32)

        def prep(dst, src_sb, w_sb, b_sb, ntok):
            for m in range(2):
                ps_t = x_ps.tile([128, 2, S], F32, tag="x")
                ps = ps_t[:, 0, :ntok]
                for k in range(2):
                    nc.tensor.matmul(
                        ps,
                        lhsT=w_sb[:, k, 128 * m : 128 * (m + 1)],
                        rhs=src_sb[:, k, :],
                        start=(k == 0),
                        stop=False,
                    )
                nc.tensor.matmul(
                    ps,
                    lhsT=b_sb[0:1, 128 * m : 128 * (m + 1)],
                    rhs=ones_sb[0:1, :ntok],
                    start=False,
                    stop=True,
                )
                nc.scalar.copy(out=dst[:, m, :], in_=ps)

        prep(qT_sb, qTin_sb, wq_sb, bq_sb, S)
        prep(kT_sb, kTin_sb, wk_sb, bk_sb, SK_LOC)
        prep(vT_sb, vTin_sb, wv_sb, bv_sb, SK_LOC)

        bvo8h_mm = const.tile([1, D], MM_DT)
        nc.vector.tensor_scalar_mul(bvo8h_mm, bvo8_sb, 0.5)
        # K=2 bias matmul operands: psum[:, m, :] += blrows[m, p] * sel[m, n]
        bl0_mm = const.tile([1, 128], MM_DT)
        bl1_mm = const.tile([1, 128], MM_DT)
        nc.vector.tensor_copy(out=bl0_mm, in_=blr0_sb)
        nc.vector.tensor_copy(out=bl1_mm, in_=blr1_sb)

        # ---- main loop over this core's keys, in groups of GRP --------------
        cc_outs = []
        vsum_ps = None  # per-half PSUM accumulator (double-buffered)
        HALF_G = SK_LOC // GRP // 2  # groups per half (loop split for RS overlap)

        for g in range(SK_LOC // GRP):
            if g % HALF_G == 0:
                vsum_ps = vs_ps.tile([128, 2, S], F32, tag="vs")
            maxe_g = cpool.tile([128, 2, GRP], F32, tag="maxe")
            ssum_g = cpool.tile([128, 2, GRP], F32, tag="ssum")
            xexps = []
            for j in range(GRP):
                sk = g * GRP + j
                # Wmod[k] = (I+Wl)[k-chunk] * k_sk  (fp16, on GpSimd)
                wmod = wmod_p.tile([128, 2, D], MM_DT, tag="wmod")
                for k in range(2):
                    nc.vector.tensor_scalar_mul(
                        wmod[:, k, :], wl_mm[:, k, :], kT_sb[:, k, sk : sk + 1]
                    )

                x_psum = x_ps.tile([128, 2, S], F32, tag="x")  # raw logits^T
                for m in range(2):
                    for k in range(2):
                        nc.tensor.matmul(
                            x_psum[:, m, :],
                            lhsT=wmod[:, k, 128 * m : 128 * (m + 1)],
                            rhs=qT_sb[:, k, :],
                            start=(k == 0),
                            stop=(k == 1),
                        )

                e_sb = spool.tile([128, 2, S], BF16, tag="e")
                xb_sb = spool.tile([128, 2, S], BF16, tag="xb")
                # bl is a per-row constant shift: it cancels out of the
                # coeff = v/(ssum+maxe) algebra exactly like C does, so the
                # exp stream can skip it and run as one fused op.
                nc.scalar.activation(
                    e_sb, x_psum, ACTF.Exp, bias=negc_sb[:], scale=1.0
                )
                for m in range(2):
                    nc.scalar.activation(
                        xb_sb[:, m, :], x_psum[:, m, :], ACTF.Identity,
                        bias=blc_sb[:, m : m + 1], scale=1.0,
                    )

                xexp_sb = xpool.tile([128, 2, S], BF16, tag="xexp")
                nc.vector.tensor_tensor(out=xexp_sb, in0=xb_sb, in1=e_sb, op=ALU.mult)

                nc.vector.tensor_reduce(
                    out=maxe_g[:, :, j : j + 1], in_=e_sb, axis=AX.X, op=ALU.max
                )
                nc.vector.tensor_reduce(
                    out=ssum_g[:, :, j : j + 1], in_=xexp_sb, axis=AX.X, op=ALU.add,
                    apply_absolute_value=True,
                )
                xexps.append(xexp_sb)

            # batched column math: coeff = v / (ssum + maxe)
            den_g = cpool.tile([128, 2, GRP], F32, tag="den")
            nc.vector.tensor_tensor(out=den_g, in0=ssum_g, in1=maxe_g, op=ALU.add)
            rec_g = cpool.tile([128, 2, GRP], F32, tag="rec")
            nc.vector.reciprocal(out=rec_g, in_=den_g)
            coeff_g = cpool.tile([128, 2, GRP], F32, tag="coeff")
            nc.vector.tensor_tensor(
                out=coeff_g, in0=rec_g,
                in1=vT_sb[:, :, g * GRP : (g + 1) * GRP], op=ALU.mult,
            )

            for j in range(GRP):
                sk = g * GRP + j
                for m in range(2):
                    diagc = dpool.tile([128, 128], BF16, tag="diag")
                    nc.scalar.mul(
                        out=diagc, in_=ident_sb, mul=coeff_g[:, m, j : j + 1]
                    )
                    nc.tensor.matmul(
                        vsum_ps[:, m, :],
                        lhsT=diagc,
                        rhs=xexps[j][:, m, :],
                        start=(sk % (SK_LOC // 2) == 0),
                        stop=(sk % (SK_LOC // 2) == SK_LOC // 2 - 1),
                    )

            if (g + 1) % HALF_G == 0:
                # drain this half's vsum, apply (I+Wvo)+bvo/8, kick off its RS
                half = (g + 1) // HALF_G - 1
                vs_sb = fpool.tile([128, 2, S], MM_DT, tag="vs")
                nc.scalar.copy(out=vs_sb[:, 0, :], in_=vsum_ps[:, 0, :])
                nc.scalar.copy(out=vs_sb[:, 1, :], in_=vsum_ps[:, 1, :])
                cc_in = dram.tile([S, D], BF16, tag=f"ccin{half}")
                for b in range(4):
                    ps_ot = x_ps.tile([128, 2, S], F32, tag="x")
                    ps_o = ps_ot[:, 0, :D]
                    for k in range(2):
                        nc.tensor.matmul(
                            ps_o,
                            lhsT=vs_sb[:, k, 128 * b : 128 * (b + 1)],
                            rhs=wvo_mm[:, k, :],
                            start=(k == 0),
                            stop=False,
                        )
                    nc.tensor.matmul(
                        ps_o,
                        lhsT=ones_mm[0:1, 0:128],
                        rhs=bvo8h_mm,
                        start=False,
                        stop=True,
                    )
                    o_sb = fpool.tile([128, D], BF16, tag="osb")
                    nc.scalar.copy(out=o_sb, in_=ps_o)
                    nc.sync.dma_start(out=cc_in[128 * b : 128 * (b + 1), :], in_=o_sb)
                if not NO_CC:
                    cc_out = dram.tile([SK_LOC, D], BF16, tag=f"ccout{half}")
                    nc.gpsimd.collective_compute(
                        "ReduceScatter",
                        ALU.add,
                        replica_groups=[list(range(N_CORES))],
                        ins=[cc_in[:].opt()],
                        outs=[cc_out[:].opt()],
                    )
                    cc_outs.append(cc_out)

        # tails were emitted inside the loop after each half
        if NO_CC:
            o32 = fpool.tile([SK_LOC, D], F32, tag="o32")
            nc.vector.memset(o32, 0.0)
            nc.sync.dma_start(out=out_ext, in_=o32)
        else:
            rs0 = fpool.tile([SK_LOC, D], BF16, tag="rsb0")
            rs1 = fpool.tile([SK_LOC, D], BF16, tag="rsb1")
            nc.sync.dma_start(out=rs0, in_=cc_outs[0][:])
            nc.sync.dma_start(out=rs1, in_=cc_outs[1][:])
            o32 = fpool.tile([SK_LOC, D], F32, tag="o32")
            nc.vector.tensor_tensor(out=o32, in0=rs0, in1=rs1, op=ALU.add)
            nc.sync.dma_start(out=out_ext, in_=o32)


def get_nc():
    if "nc" not in _CACHE:
        _CACHE["nc"] = _build()
    return _CACHE["nc"]


def make_in_maps(inputs):
    """Host-side prep: transposes, residual weight folding, Sk sharding."""
    f32 = np.float32
    q = np.ascontiguousarray(inputs["query_tokens"][0].T, dtype=f32)  # [D,S]
    kT = np.ascontiguousarray(inputs["key_tokens"][0].T, dtype=f32)
    vT = np.ascontiguousarray(inputs["value_tokens"][0].T, dtype=f32)
    eye = np.eye(D, dtype=f32)
    wq = np.ascontiguousarray(eye + inputs["Wq"], dtype=f32)
    wk = np.ascontiguousarray(eye + inputs["Wk"], dtype=f32)
    wv = np.ascontiguousarray(eye + inputs["Wva"], dtype=f32)
    wl = np.ascontiguousarray(eye + inputs["Wl"], dtype=f32)
    wvo = np.ascontiguousarray(eye + inputs["Wvo"], dtype=f32)
    ident = np.eye(128, dtype=f32).astype(ml_dtypes.bfloat16)

    base = {
        "qTin": q,
        "wq": wq,
        "wk": wk,
        "wv": wv,
        "wl": wl,
        "wvo": wvo,
        "bq": inputs["bq"].reshape(1, D).astype(f32),
        "bk": inputs["bk"].reshape(1, D).astype(f32),
        "bv": inputs["bva"].reshape(1, D).astype(f32),
        "blc": np.ascontiguousarray(
            inputs["bl"].reshape(2, 128).T, dtype=f32
        ),  # [128,2]: bias column per dout chunk
        "blr": np.ascontiguousarray(inputs["bl"].reshape(2, 128), dtype=f32),
        "bvo8": (inputs["bvo"].reshape(1, D) / N_CORES).astype(f32),
        "ident": ident,
    }
    in_maps = []
    for c in range(N_CORES):
        m = dict(base)
        sl = slice(c * SK_LOC, (c + 1) * SK_LOC)
        m["kTin"] = np.ascontiguousarray(kT[:, sl], dtype=f32)
        m["vTin"] = np.ascontiguousarray(vT[:, sl], dtype=f32)
        in_maps.append(m)
    return in_maps


def kernel(**inputs):
    nc = get_nc()
    in_maps = make_in_maps(inputs)
    res = run_bass_kernel_spmd(nc, in_maps, core_ids=list(range(N_CORES)))
    out = np.concatenate([res.results[c]["out"] for c in range(N_CORES)], axis=0)
    return out.reshape(1, S, D).astype(np.float32)
